# revision 1
# baseline (speedup 1.0000x reference)
"""CenterLoss kernel for Trainium2 (8 NeuronCores, data-parallel over batch).

reference:  mean(clip(rowsum((x - labels @ centers)^2), 1e-12, 1e12))
labels are exact one-hot rows, so labels @ centers is an embedding gather:
    idx[b]  = sum_j labels[b, j] * j            (exact in f32, idx < 751)
    c[b]    = centers[idx[b], :]                (indirect DMA row gather)
    ps[b]   = rowsum((x[b] - c[b])^2)           (DVE sub + ACT square+accum)
Per-core output is the [128, 8] tile of per-sample sums; the host applies
the clip (never binding for this data, but exact) and takes the mean.
"""

import numpy as np

import concourse.bacc as bacc
import concourse.bass as bass
import concourse.mybir as mybir
from concourse.tile import TileContext
from concourse.bass_utils import run_bass_kernel_spmd

F32 = mybir.dt.float32
I32 = mybir.dt.int32

NCORES = 8
B = 8192          # full batch
C = 751           # num classes
D = 2048          # feature dim
BS = B // NCORES  # batch per core = 1024
P = 128           # partitions
NT = BS // P      # batch tiles per core = 8

CLIP_LO, CLIP_HI = 1e-12, 1e12


def build_nc():
    nc = bacc.Bacc(
        "TRN2",
        target_bir_lowering=False,
        debug=False,
        num_devices=NCORES,
    )
    x = nc.dram_tensor("x", [BS, D], F32, kind="ExternalInput")
    labels = nc.dram_tensor("labels", [BS, C], F32, kind="ExternalInput")
    centers = nc.dram_tensor("centers", [C, D], F32, kind="ExternalInput")
    out = nc.dram_tensor("out", [P, NT], F32, kind="ExternalOutput")

    with TileContext(nc) as tc:
        with (
            tc.tile_pool(name="const", bufs=1) as cpool,
            tc.tile_pool(name="lab", bufs=3) as lpool,
            tc.tile_pool(name="xs", bufs=1) as xpool,
            tc.tile_pool(name="dif", bufs=2) as dpool,
            tc.tile_pool(name="dsq", bufs=1) as sqpool,
            tc.tile_pool(name="ctr", bufs=1) as ctrpool,
            tc.tile_pool(name="small", bufs=1) as spool,
        ):
            # 0..C-1 along the free dim, identical in every partition
            iota_f = cpool.tile([P, C], F32)
            nc.gpsimd.iota(
                iota_f[:],
                pattern=[[1, C]],
                base=0,
                channel_multiplier=0,
                allow_small_or_imprecise_dtypes=True,
            )

            idx_f = spool.tile([P, NT], F32)
            idx_i = spool.tile([P, NT], I32)
            acc = spool.tile([P, NT], F32)
            ctile = ctrpool.tile([P, NT, D], F32)
            # all tiles live at once -> DMAs never stall on slot reuse
            xbig = xpool.tile([P, NT, D], F32)
            lbig = xpool.tile([P, NT, C], F32)

            labels_r = labels.rearrange("(n p) c -> p n c", p=P)
            x_r = x.rearrange("(n p) d -> p n d", p=P)
            with tc.high_priority():
                # labels stream alone on the sync HWDGE ring, 2 large DMAs
                # (fewer DMAs -> fewer completion gaps -> higher HBM rate)
                for g in range(2):
                    h = NT // 2
                    nc.sync.dma_start(
                        out=lbig[:, g * h:(g + 1) * h, :],
                        in_=labels_r[:, g * h:(g + 1) * h, :],
                    )

            # x loads on the scalar HWDGE ring (separate FIFO from labels),
            # 4 chunks of 2 batch tiles each
            for g in range(4):
                h = NT // 4
                nc.scalar.dma_start(
                    out=xbig[:, g * h:(g + 1) * h, :],
                    in_=x_r[:, g * h:(g + 1) * h, :],
                )

            with tc.high_priority(offset=10**6):
                for n in range(NT):
                    prod = lpool.tile([P, C], F32)
                    # idx_f[:, n] = rowsum(lab * iota)
                    nc.vector.tensor_mul(
                        out=prod[:], in0=lbig[:, n, :], in1=iota_f[:]
                    )
                    nc.vector.reduce_sum(
                        out=idx_f[:, n:n + 1], in_=prod[:],
                        axis=mybir.AxisListType.X,
                    )
                    nc.vector.tensor_copy(
                        out=idx_i[:, n:n + 1], in_=idx_f[:, n:n + 1]
                    )
                    # ctile[p, n, :] = centers[idx[p, n], :]
                    # (HW supports one offset per partition per indirect DMA)
                    nc.gpsimd.indirect_dma_start(
                        out=ctile[:, n, :],
                        out_offset=None,
                        in_=centers[:],
                        in_offset=bass.IndirectOffsetOnAxis(
                            ap=idx_i[:, n:n + 1], axis=0
                        ),
                    )

            for n in range(NT):
                dif = dpool.tile([P, D], F32)
                nc.vector.tensor_sub(
                    out=dif[:], in0=xbig[:, n, :], in1=ctile[:, n, :]
                )
                dsq = sqpool.tile([P, D], F32)
                nc.scalar.activation(
                    out=dsq[:],
                    in_=dif[:],
                    func=mybir.ActivationFunctionType.Square,
                    accum_out=acc[:, n:n + 1],
                )

            nc.sync.dma_start(out=out[:], in_=acc[:])

    nc.compile()
    return nc


_NC = None


def _get_nc():
    global _NC
    if _NC is None:
        _NC = build_nc()
    return _NC


def run_sharded(inputs: dict, trace: bool = False):
    """Shard, run on 8 cores, return (per_sample [B] f32, BassKernelResults)."""
    x = np.ascontiguousarray(np.asarray(inputs["x"], dtype=np.float32))
    labels = np.ascontiguousarray(np.asarray(inputs["labels"], dtype=np.float32))
    centers = np.ascontiguousarray(np.asarray(inputs["centers"], dtype=np.float32))
    assert x.shape == (B, D) and labels.shape == (B, C) and centers.shape == (C, D)

    in_maps = [
        {
            "x": np.ascontiguousarray(x[k * BS:(k + 1) * BS]),
            "labels": np.ascontiguousarray(labels[k * BS:(k + 1) * BS]),
            "centers": centers,
        }
        for k in range(NCORES)
    ]
    res = run_bass_kernel_spmd(
        _get_nc(), in_maps, core_ids=list(range(NCORES)), trace=trace
    )
    # out[p, n] holds sample k*BS + n*P + p
    per_sample = np.concatenate(
        [res.results[k]["out"].T.reshape(-1) for k in range(NCORES)]
    )
    return per_sample, res


def kernel(x, labels, centers):
    per_sample, _ = run_sharded({"x": x, "labels": labels, "centers": centers})
    per_sample = np.clip(per_sample, CLIP_LO, CLIP_HI)
    return np.asarray(per_sample.mean(dtype=np.float64), dtype=np.float32)

